# revision 1
# baseline (speedup 1.0000x reference)
"""Trainium2 Bass kernel for nn_CNNGRUforHorizon (CNN+BiGRU audio model).

Strategy: W-shard the logmel branch + fusion conv across 8 cores (each core
owns 64 of the 512 fused-map columns, with halo), replicate the tiny wave
branch, AllReduce the 64KB feature matrix, then run the 32-step BiGRU
replicated on every core. Convolutions and the GRU recurrence matmuls run
as float32r; accumulation stays fp32.
"""
import os
import sys

import numpy as np


def _ensure_concourse():
    try:
        import concourse  # noqa: F401
        return
    except ImportError:
        pass
    for p in ("/opt/trn_rl_repo", "/root/.axon_site/_ro/trn_rl_repo"):
        if os.path.isdir(p) and p not in sys.path:
            sys.path.insert(0, p)
    import concourse  # noqa: F401


NCORES = 8
LAST_RESULTS = None
_CACHE = {}


def _resize_matrix(n_in, n_out):
    R = np.zeros((n_in, n_out), np.float64)
    for x in range(n_out):
        c = (x + 0.5) * n_in / n_out - 0.5
        i0 = int(np.floor(c))
        w1 = c - i0
        i0c = min(max(i0, 0), n_in - 1)
        i1c = min(max(i0 + 1, 0), n_in - 1)
        R[i0c, x] += 1.0 - w1
        R[i1c, x] += w1
    return R.astype(np.float32)


def _build():
    import concourse.bacc as bacc
    import concourse.mybir as mybir
    import concourse.tile as tile

    f32 = mybir.dt.float32
    f32r = mybir.dt.float32r
    AF = mybir.ActivationFunctionType
    ALU = mybir.AluOpType
    AX = mybir.AxisListType

    nc = bacc.Bacc("TRN2", target_bir_lowering=False, debug=False,
                   num_devices=NCORES)

    def din(name, shape, dt=f32r):
        return nc.dram_tensor(name, shape, dt, kind="ExternalInput")

    lmI_d = din("lmI", [9, 8576])            # logmel im2col slice, per core
    wP_d = din("wP", [5, 3203])              # wave, stride-5 im2col planes
    Ri_d = din("Ri", [50, 66])               # resize matrix slice, per core
    msk_d = din("msk", [128, 2], f32)        # x_mel edge-col masks, per core
    zed_d = din("zed", [128, 2], f32)        # zeros (GRU h0 via warmup CC)

    w1m_d = din("w1m", [9, 64])
    bn1s_d = din("bn1s", [64, 1], f32)
    bn1b_d = din("bn1b", [64, 1], f32)
    w2m_d = din("w2m", [128, 6, 128])
    bn2s_d = din("bn2s", [128, 1], f32)
    bn2b_d = din("bn2b", [128, 1], f32)
    w3m_d = din("w3m", [128, 2, 128])
    b3_d = din("b3", [128, 2], f32)
    fcm_d = din("fcm", [128, 2, 9, 4, 128])      # fc mel-half weights
    fcw_d = din("fcw", [128, 2, 3, 3, 4, 128])   # fc wave-half (3 variants)
    fb4_d = din("fb4", [128, 4], f32)

    w1w_d = din("w1w", [5, 3, 64])
    wb1_d = din("wb1", [64, 1], f32)
    w2w_d = din("w2w", [64, 5, 128])
    wb2_d = din("wb2", [128, 1], f32)
    w3w_d = din("w3w", [128, 5, 2, 128])
    wb3w_d = din("wb3w", [128, 2], f32)
    w4wT_d = din("w4wT", [128, 2, 5, 256])
    wb4row_d = din("wb4row", [1, 256])
    one50_d = din("one50", [1, 50])

    wihT_d = din("wihT", [128, 2, 3, 4, 128])    # f32r
    brz_d = din("brz", [128, 2, 2], f32)
    bng_d = din("bng", [128, 2], f32)
    bhhn_d = din("bhhn", [128, 2], f32)
    whhT_d = din("whhT", [128, 2, 3, 128])       # f32r
    clsT_d = din("clsT", [128, 2, 5], f32)
    clsb5_d = din("clsb5", [5, 1], f32)

    out_d = nc.dram_tensor("out", [1, 5], f32, kind="ExternalOutput")

    with tile.TileContext(nc) as tc:
        with (
            tc.tile_pool(name="keep", bufs=1) as keep,      # long-lived
            tc.tile_pool(name="psum", bufs=1, space="PSUM") as psp,
            tc.tile_pool(name="dram", bufs=1, space="DRAM") as dram,
            tc.tile_pool(name="sc", bufs=1) as sc,          # small scratch
        ):
            xwr8 = keep.tile([128, 2, 8, 66], f32r)
            msk = keep.tile([128, 2], f32)
            nc.sync.dma_start(msk[:], msk_d[:])
            featp = keep.tile([128, 4, 32], f32)
            fb4 = keep.tile([128, 4], f32)
            nc.sync.dma_start(fb4[:], fb4_d[:])

            def psum_mm(shape):
                return psp.tile(shape, f32, tag="mm", bufs=3, name="psmm")

            def psum_aux(shape):
                return psp.tile(shape, f32, tag="aux", bufs=4, name="psaux")

            # warmup collective early (absorbs CC engine startup); its
            # output (zeros) becomes the GRU initial hidden state.
            ccz_i = dram.tile([128, 2], f32)
            ccz_o = dram.tile([128, 2], f32)
            nc.sync.dma_start(ccz_i[:], zed_d[:])
            nc.gpsimd.collective_compute(
                "AllReduce", ALU.add,
                replica_groups=[list(range(NCORES))],
                ins=[ccz_i.opt()], outs=[ccz_o.opt()])

            # ============== WAVE BRANCH (replicated) =====================
            with tc.tile_pool(name="wave", bufs=1) as wv:
                Pt = wv.tile([5, 3203], f32r)
                nc.sync.dma_start(Pt[:], wP_d[:])
                w1w = wv.tile([5, 3, 64], f32r)
                nc.sync.dma_start(w1w[:], w1w_d[:])
                wb1 = wv.tile([64, 1], f32)
                nc.sync.dma_start(wb1[:], wb1_d[:])
                w2w = wv.tile([64, 5, 128], f32r)
                nc.sync.dma_start(w2w[:], w2w_d[:])
                wb2 = wv.tile([128, 1], f32)
                nc.sync.dma_start(wb2[:], wb2_d[:])
                w3w = wv.tile([128, 5, 2, 128], f32r)
                nc.sync.dma_start(w3w[:], w3w_d[:])
                wb3w = wv.tile([128, 2], f32)
                nc.sync.dma_start(wb3w[:], wb3w_d[:])
                w4wT = wv.tile([128, 2, 5, 256], f32r)
                nc.sync.dma_start(w4wT[:], w4wT_d[:])
                wb4row = wv.tile([1, 256], f32r)
                nc.sync.dma_start(wb4row[:], wb4row_d[:])
                one50 = wv.tile([1, 50], f32r)
                nc.sync.dma_start(one50[:], one50_d[:])
                Ri = wv.tile([50, 66], f32r)
                nc.sync.dma_start(Ri[:], Ri_d[:])

                # conv1: 16000 -> 3200, k=11 s=5 via 3 taps of K=5
                w1o = wv.tile([64, 3200], f32r)
                for c in range(7):
                    n0 = 512 * c
                    n = min(512, 3200 - n0)
                    ps = psum_aux([64, 512])
                    for m in range(3):
                        nc.tensor.matmul(ps[:, :n], w1w[:, m, :],
                                         Pt[:, m + n0:m + n0 + n],
                                         start=(m == 0), stop=(m == 2))
                    nc.scalar.activation(w1o[:, n0:n0 + n], ps[:, :n],
                                         AF.Relu, bias=wb1[:, 0:1])
                p1t = wv.tile([64, 804], f32r)
                nc.vector.memset(p1t[:, 0:2].bitcast(f32), 0.0)
                nc.vector.memset(p1t[:, 802:804].bitcast(f32), 0.0)
                t1 = wv.tile([64, 800], f32r)
                t2 = wv.tile([64, 800], f32r)
                nc.vector.tensor_tensor(t1[:], w1o[:, 0:3200:4],
                                        w1o[:, 1:3200:4], op=ALU.max)
                nc.vector.tensor_tensor(t2[:], w1o[:, 2:3200:4],
                                        w1o[:, 3:3200:4], op=ALU.max)
                nc.vector.tensor_tensor(p1t[:, 2:802], t1[:], t2[:],
                                        op=ALU.max)
                # conv2: k=5 pad 2, 64 -> 128 ch, 800 cols
                w2o = wv.tile([128, 800], f32r)
                for c in range(2):
                    n0 = 512 * c
                    n = min(512, 800 - n0)
                    ps = psum_aux([128, 512])
                    for tap in range(5):
                        nc.tensor.matmul(ps[:, :n], w2w[:, tap, :],
                                         p1t[:, n0 + tap:n0 + tap + n],
                                         start=(tap == 0), stop=(tap == 4))
                    nc.scalar.activation(w2o[:, n0:n0 + n], ps[:, :n],
                                         AF.Relu, bias=wb2[:, 0:1])
                p2t = wv.tile([128, 204], f32r)
                nc.vector.memset(p2t[:, 0:2].bitcast(f32), 0.0)
                nc.vector.memset(p2t[:, 202:204].bitcast(f32), 0.0)
                t3 = wv.tile([128, 200], f32r)
                t4 = wv.tile([128, 200], f32r)
                nc.vector.tensor_tensor(t3[:], w2o[:, 0:800:4],
                                        w2o[:, 1:800:4], op=ALU.max)
                nc.vector.tensor_tensor(t4[:], w2o[:, 2:800:4],
                                        w2o[:, 3:800:4], op=ALU.max)
                nc.vector.tensor_tensor(p2t[:, 2:202], t3[:], t4[:],
                                        op=ALU.max)
                # conv3: k=5 pad 2, 128 -> 256 ch, 200 cols
                w3o = wv.tile([128, 2, 200], f32r)
                for oc in range(2):
                    ps = psum_aux([128, 512])
                    for tap in range(5):
                        nc.tensor.matmul(ps[:, :200], w3w[:, tap, oc, :],
                                         p2t[:, tap:tap + 200],
                                         start=(tap == 0), stop=(tap == 4))
                    nc.scalar.activation(w3o[:, oc, :], ps[:, :200],
                                         AF.Relu, bias=wb3w[:, oc:oc + 1])
                p3t = wv.tile([128, 2, 54], f32r)
                for oc in range(2):
                    nc.vector.memset(p3t[:, oc, 0:2].bitcast(f32), 0.0)
                    nc.vector.memset(p3t[:, oc, 52:54].bitcast(f32), 0.0)
                    t5 = wv.tile([128, 50], f32r, tag="t5")
                    t6 = wv.tile([128, 50], f32r, tag="t6")
                    nc.vector.tensor_tensor(t5[:], w3o[:, oc, 0:200:4],
                                            w3o[:, oc, 1:200:4], op=ALU.max)
                    nc.vector.tensor_tensor(t6[:], w3o[:, oc, 2:200:4],
                                            w3o[:, oc, 3:200:4], op=ALU.max)
                    nc.vector.tensor_tensor(p3t[:, oc, 2:52], t5[:], t6[:],
                                            op=ALU.max)
                # conv4 (transposed out): k=5 pad 2, 256 -> 256 ch, 50 cols
                ps4 = psum_aux([50, 256])
                first = True
                for ch in range(2):
                    for tap in range(5):
                        nc.tensor.matmul(ps4[:], p3t[:, ch, tap:tap + 50],
                                         w4wT[:, ch, tap, :],
                                         start=first, stop=False)
                        first = False
                nc.tensor.matmul(ps4[:], one50[:], wb4row[:],
                                 start=False, stop=True)
                xwT = wv.tile([50, 256], f32r)
                nc.scalar.activation(xwT[:], ps4[:], AF.Relu)
                # resize 50 -> local 66 cols (per-core R slice)
                for oc in range(2):
                    psR = psum_aux([128, 66])
                    nc.tensor.matmul(psR[:], xwT[:, 128 * oc:128 * (oc + 1)],
                                     Ri[:], start=True, stop=True)
                    for r in range(8):
                        nc.scalar.activation(xwr8[:, oc, r, :], psR[:],
                                             AF.Identity)

            # ============== MEL BRANCH (W-sharded) =======================
            with tc.tile_pool(name="melw", bufs=1) as mw:
                w1m = mw.tile([9, 64], f32r)
                nc.sync.dma_start(w1m[:], w1m_d[:])
                bn1s = mw.tile([64, 1], f32)
                nc.sync.dma_start(bn1s[:], bn1s_d[:])
                bn1b = mw.tile([64, 1], f32)
                nc.sync.dma_start(bn1b[:], bn1b_d[:])
                w2m = mw.tile([128, 6, 128], f32r)
                nc.sync.dma_start(w2m[:], w2m_d[:])
                bn2s = mw.tile([128, 1], f32)
                nc.sync.dma_start(bn2s[:], bn2s_d[:])
                bn2b = mw.tile([128, 1], f32)
                nc.sync.dma_start(bn2b[:], bn2b_d[:])
                w3m = mw.tile([128, 2, 128], f32r)
                nc.sync.dma_start(w3m[:], w3m_d[:])
                b3 = mw.tile([128, 2], f32)
                nc.sync.dma_start(b3[:], b3_d[:])

                xmel = keep.tile([128, 2, 34, 66], f32r)
                for oc in range(2):
                    nc.vector.memset(xmel[:, oc, 0, :].bitcast(f32), 0.0)
                    nc.vector.memset(xmel[:, oc, 33, :].bitcast(f32), 0.0)

                # lifetimes: rhs1 < m1 < m2 < m3 (p_m1 allocated first so
                # p_rhs can release right after the m1 matmuls, LIFO-safe)
                p_m1 = tc.alloc_tile_pool(name="mel_m1", bufs=1)
                p_rhs = tc.alloc_tile_pool(name="mel_rhs", bufs=1)
                rhs1 = p_rhs.tile([9, 8576], f32r)
                nc.sync.dma_start(rhs1[:], lmI_d[:])
                # m1 flat [128, 8848]: partitions 0-63 hold row slot r at
                # offset 134*r (slots 0..65); partitions 64-127 hold the
                # same data shifted one row (for K=128 dy-packed m2 matmuls)
                m1 = p_m1.tile([128, 8848], f32r, tag="m1")
                nc.vector.memset(m1[0:64, 0:134].bitcast(f32), 0.0)
                nc.vector.memset(m1[0:64, 8710:8848].bitcast(f32), 0.0)
                nc.vector.memset(m1[64:128, 8714:8848].bitcast(f32), 0.0)
                for c in range(17):
                    n0 = 512 * c
                    n = min(512, 8576 - n0)
                    ps = psum_mm([64, 512])
                    nc.tensor.matmul(ps[:, :n], w1m[:], rhs1[:, n0:n0 + n],
                                     start=True, stop=True)
                    if c % 2 == 0:
                        nc.scalar.activation(
                            m1[0:64, 134 + n0:134 + n0 + n],
                            ps[:, :n], AF.Relu,
                            bias=bn1b[:, 0:1], scale=bn1s[:, 0:1])
                    else:
                        nc.vector.tensor_scalar(
                            m1[0:64, 134 + n0:134 + n0 + n],
                            ps[:, :n], bn1s[:, 0:1], bn1b[:, 0:1],
                            op0=ALU.mult, op1=ALU.add)
                        nc.vector.tensor_scalar_max(
                            m1[0:64, 134 + n0:134 + n0 + n],
                            m1[0:64, 134 + n0:134 + n0 + n], 0.0)
                    eng = nc.gpsimd if c % 2 == 0 else nc.sync
                    eng.dma_start(m1[64:128, n0:n0 + n],
                                  m1[0:64, 134 + n0:134 + n0 + n])

                p_rhs.release()
                # m2: 3x3 conv, 64 -> 128 ch, flat 64x134 grid (2 junk cols)
                p_m2 = tc.alloc_tile_pool(name="mel_m2", bufs=1)
                m2 = p_m2.tile([128, 8576], f32r)
                for c in range(17):
                    n0 = 512 * c
                    n = min(512, 8576 - n0)
                    ps = psum_mm([128, 512])
                    for j in range(6):
                        off = (268 if j >= 3 else 0) + (j % 3) + n0
                        nc.tensor.matmul(
                            ps[:, :n], w2m[:, j, :],
                            m1[:, off:off + n],
                            start=(j == 0), stop=(j == 5))
                    nc.scalar.activation(m2[:, n0:n0 + n],
                                         ps[:, :n], AF.Relu,
                                         bias=bn2b[:, 0:1],
                                         scale=bn2s[:, 0:1])

                # m3 (1x1 conv, 128 -> 256) + maxpool 2x2 per chunk
                p_m3 = tc.alloc_tile_pool(name="mel_m3", bufs=1)
                for oc in range(2):
                    m3 = p_m3.tile([128, 8576], f32r, tag="m3",
                                   bufs=2, name="m3")
                    for c in range(17):
                        n0 = 512 * c
                        n = min(512, 8576 - n0)
                        ps = psum_mm([128, 512])
                        nc.tensor.matmul(ps[:, :n], w3m[:, oc, :],
                                         m2[:, n0:n0 + n],
                                         start=True, stop=True)
                        nc.scalar.activation(
                            m3[:, n0:n0 + n], ps[:, :n], AF.Relu,
                            bias=b3[:, oc:oc + 1])
                    m3v = m3.rearrange("p (a b) -> p a b", b=134)
                    vp = p_m3.tile([128, 32, 134], f32r, tag="vp",
                                   bufs=1, name="vp")
                    nc.vector.tensor_tensor(
                        vp[:], m3v[:, 0:64:2, :], m3v[:, 1:64:2, :],
                        op=ALU.max)
                    nc.vector.tensor_tensor(
                        xmel[:, oc, 1:33, :], vp[:, :, 0:132:2],
                        vp[:, :, 1:132:2], op=ALU.max)
                for j, col in ((0, 0), (1, 65)):
                    nc.vector.tensor_scalar_mul(
                        xmel[:, :, 1:33, col:col + 1],
                        xmel[:, :, 1:33, col:col + 1],
                        msk[:, j:j + 1])
                p_m3.release()
                p_m2.release()
                p_m1.release()

            # GRU weights/buffers pool (small, lives to the end)
            p_gru = tc.alloc_tile_pool(name="gru", bufs=1)
            wihT = p_gru.tile([128, 2, 3, 4, 128], f32r)
            nc.sync.dma_start(wihT[:], wihT_d[:])
            brz = p_gru.tile([128, 2, 2], f32)
            nc.sync.dma_start(brz[:], brz_d[:])
            bng = p_gru.tile([128, 2], f32)
            nc.sync.dma_start(bng[:], bng_d[:])
            bhhn = p_gru.tile([128, 2], f32)
            nc.sync.dma_start(bhhn[:], bhhn_d[:])
            whhT = p_gru.tile([128, 2, 3, 128], f32r)
            nc.sync.dma_start(whhT[:], whhT_d[:])
            clsT = p_gru.tile([128, 2, 5], f32)
            nc.sync.dma_start(clsT[:], clsT_d[:])
            clsb5 = p_gru.tile([5, 1], f32)
            nc.sync.dma_start(clsb5[:], clsb5_d[:])
            ggxrz = p_gru.tile([128, 2, 32, 2], f32)
            ggxn = p_gru.tile([128, 2, 32], f32)
            ft = p_gru.tile([128, 4, 32], f32)
            ftr = p_gru.tile([128, 4, 32], f32r)
            # hall[:, d, s, 0]: hidden state per step (col 1 stays zero --
            # fp32r matmuls need an even moving-dim count)
            hall = p_gru.tile([128, 2, 33, 2], f32r)
            nc.vector.memset(hall[:].bitcast(f32), 0.0)
            hsum = p_gru.tile([128, 2], f32)

            # ============== FC FUSION CONV + feat ========================
            # per-oc weight chunks, double-buffered: streams behind compute
            with tc.tile_pool(name="fcwp", bufs=1) as fw:
                for oc in range(4):
                    fcm = fw.tile([128, 2, 9, 128], f32r, tag="fcm", bufs=2,
                                  name="fcm")
                    nc.sync.dma_start(fcm[:], fcm_d[:, :, :, oc, :])
                    fcw = fw.tile([128, 2, 3, 3, 128], f32r, tag="fcwt",
                                  bufs=2, name="fcw")
                    nc.sync.dma_start(fcw[:], fcw_d[:, :, :, :, oc, :])
                    for rg in range(4):
                        ps = psum_aux([128, 8, 64])
                        first = True
                        for ch in range(2):
                            for dy in range(3):
                                for dx in range(3):
                                    nc.tensor.matmul(
                                        ps[:],
                                        fcm[:, ch, 3 * dy + dx, :],
                                        xmel[:, ch, rg * 8 + dy:
                                             rg * 8 + dy + 8, dx:dx + 64],
                                        start=first, stop=False)
                                    first = False
                        for ch in range(2):
                            for dx in range(3):
                                last = (ch == 1 and dx == 2)
                                if rg == 0:
                                    nc.tensor.matmul(
                                        ps[:, 0:1, :],
                                        fcw[:, ch, 1, dx, :],
                                        xwr8[:, ch, 0:1, dx:dx + 64],
                                        start=False, stop=False)
                                    nc.tensor.matmul(
                                        ps[:, 1:8, :],
                                        fcw[:, ch, 0, dx, :],
                                        xwr8[:, ch, 0:7, dx:dx + 64],
                                        start=False, stop=last)
                                elif rg == 3:
                                    nc.tensor.matmul(
                                        ps[:, 0:7, :],
                                        fcw[:, ch, 0, dx, :],
                                        xwr8[:, ch, 0:7, dx:dx + 64],
                                        start=False, stop=False)
                                    nc.tensor.matmul(
                                        ps[:, 7:8, :],
                                        fcw[:, ch, 2, dx, :],
                                        xwr8[:, ch, 0:1, dx:dx + 64],
                                        start=False, stop=last)
                                else:
                                    nc.tensor.matmul(
                                        ps[:],
                                        fcw[:, ch, 0, dx, :],
                                        xwr8[:, ch, :, dx:dx + 64],
                                        start=False, stop=last)
                        xft = sc.tile([128, 8, 64], f32r, tag="xf", bufs=3)
                        nc.scalar.activation(xft[:], ps[:], AF.Relu,
                                             bias=fb4[:, oc:oc + 1])
                        nc.vector.tensor_reduce(
                            featp[:, oc, rg * 8:rg * 8 + 8], xft[:],
                            axis=AX.X, op=ALU.add)

            # ============== ALLREDUCE feat ===============================
            ccin = dram.tile([512, 32], f32)
            ccout = dram.tile([512, 32], f32)
            for oc in range(4):
                nc.sync.dma_start(ccin[128 * oc:128 * (oc + 1), :],
                                  featp[:, oc, :])
            nc.gpsimd.collective_compute(
                "AllReduce", ALU.add,
                replica_groups=[list(range(NCORES))],
                ins=[ccin.opt()], outs=[ccout.opt()])
            for oc in range(4):
                nc.sync.dma_start(ft[:, oc, :],
                                  ccout[128 * oc:128 * (oc + 1), :])
            nc.vector.tensor_copy(ftr[:], ft[:])
            # h0 (zeros) arrives from the warmup collective
            nc.sync.dma_start(hall[:, :, 0:1, 0:1].bitcast(f32), ccz_o[:])

            # ============== GRU (replicated) =============================
            for d in range(2):
                for g in range(3):
                    ps = psum_aux([128, 32])
                    for kk in range(4):
                        nc.tensor.matmul(ps[:], wihT[:, d, g, kk, :],
                                         ftr[:, kk, :],
                                         start=(kk == 0), stop=(kk == 3))
                    if g < 2:
                        nc.scalar.activation(ggxrz[:, d, :, g], ps[:],
                                             AF.Identity,
                                             bias=brz[:, d, g:g + 1],
                                             scale=(-1.0 if g == 1 else 1.0))
                    else:
                        nc.scalar.activation(ggxn[:, d, :], ps[:],
                                             AF.Identity,
                                             bias=bng[:, d:d + 1])

            for s in range(32):
                for d in range(2):
                    t = s if d == 0 else 31 - s
                    ps = psum_aux([128, 3, 2])
                    for g in range(3):
                        nc.tensor.matmul(ps[:, g, :], whhT[:, d, g, :],
                                         hall[:, d, s, :],
                                         start=True, stop=True)
                    rz = sc.tile([128, 2], f32, tag="rz", bufs=4)
                    nc.scalar.activation(rz[:, 0:1], ps[:, 0, 0:1],
                                         AF.Sigmoid,
                                         bias=ggxrz[:, d, t, 0:1])
                    nc.scalar.activation(rz[:, 1:2], ps[:, 1, 0:1],
                                         AF.Sigmoid, scale=-1.0,
                                         bias=ggxrz[:, d, t, 1:2])
                    tn = sc.tile([128, 1], f32, tag="tn", bufs=4)
                    nc.vector.scalar_tensor_tensor(
                        tn[:], ps[:, 2, 0:1], bhhn[:, d:d + 1], rz[:, 0:1],
                        op0=ALU.add, op1=ALU.mult)
                    nt = sc.tile([128, 1], f32, tag="nt", bufs=4)
                    nc.scalar.activation(nt[:], tn[:], AF.Tanh,
                                         bias=ggxn[:, d, t:t + 1])
                    # rz[:,1] holds z' = 1-z.  t1 = h*z' - h (off critical
                    # path); h' = n*z' - t1 = (1-z)*n + z*h (one op after tanh)
                    hmn = sc.tile([128, 1], f32, tag="hmn", bufs=4)
                    nc.vector.scalar_tensor_tensor(
                        hmn[:], hall[:, d, s, 0:1], rz[:, 1:2],
                        hall[:, d, s, 0:1], op0=ALU.mult, op1=ALU.subtract)
                    nc.vector.scalar_tensor_tensor(
                        hall[:, d, s + 1, 0:1], nt[:], rz[:, 1:2], hmn[:],
                        op0=ALU.mult, op1=ALU.subtract)

            nc.vector.tensor_reduce(hsum[:], hall[:, :, 1:33, 0],
                                    axis=AX.X, op=ALU.add)
            psc = psum_aux([5, 1])
            for d in range(2):
                nc.tensor.matmul(psc[:], clsT[:, d, :], hsum[:, d:d + 1],
                                 start=(d == 0), stop=(d == 1))
            lgt = sc.tile([5, 1], f32, tag="lgt")
            nc.scalar.activation(lgt[:], psc[:], AF.Identity,
                                 bias=clsb5[:, 0:1])
            nc.sync.dma_start(out_d[0:1, :].rearrange("a p -> p a"), lgt[:])
            p_gru.release()

    nc.compile()
    return nc


def _prep_inputs(inputs):
    """Build the 8 per-core input maps from the full model inputs."""
    f = np.float32
    wave = np.asarray(inputs["waveform"], f).reshape(16000)
    logmel = np.asarray(inputs["logmel"], f).reshape(64, 1024)

    wp = np.zeros(16015, f)
    wp[3:16003] = wave
    wP = np.ascontiguousarray(wp.reshape(3203, 5).T)   # [5, 3203]

    R = _resize_matrix(50, 512)
    Rp = np.zeros((50, 514), f)
    Rp[:, 1:513] = R

    lmp = np.pad(logmel, ((1, 1), (4, 4)))

    w1m = np.ascontiguousarray(
        np.asarray(inputs["mc1"], f).reshape(64, 9).T)
    s1 = np.asarray(inputs["bn1g"], f) / np.sqrt(
        np.asarray(inputs["bn1v"], f) + 1e-5)
    b1 = (np.asarray(inputs["mb1"], f) - np.asarray(inputs["bn1m"], f)) * s1 \
        + np.asarray(inputs["bn1b"], f)
    mc2 = np.asarray(inputs["mc2"], f)              # [128, 64, 3, 3]
    w2m = np.zeros((128, 6, 128), f)
    for dx in range(3):
        w2m[0:64, dx, :] = mc2[:, :, 0, dx].T
        w2m[64:128, dx, :] = mc2[:, :, 1, dx].T
        w2m[0:64, 3 + dx, :] = mc2[:, :, 2, dx].T
    s2 = np.asarray(inputs["bn2g"], f) / np.sqrt(
        np.asarray(inputs["bn2v"], f) + 1e-5)
    b2 = (np.asarray(inputs["mb2"], f) - np.asarray(inputs["bn2m"], f)) * s2 \
        + np.asarray(inputs["bn2b"], f)
    w3m = np.ascontiguousarray(
        np.asarray(inputs["mc3"], f).reshape(256, 128).T.reshape(128, 2, 128))
    b3 = np.ascontiguousarray(
        np.asarray(inputs["mb3"], f).reshape(2, 128).T)

    fc = np.asarray(inputs["fc"], f)                   # [512,512,3,3]
    fcmel = fc[:, 256:, :, :]
    fcm = np.ascontiguousarray(
        fcmel.reshape(4, 128, 2, 128, 9).transpose(3, 2, 4, 0, 1))
    fcwave = fc[:, :256, :, :]
    wsum = np.stack([
        fcwave.sum(axis=2),
        fcwave[:, :, 1:, :].sum(axis=2),
        fcwave[:, :, :2, :].sum(axis=2),
    ], axis=2)                              # [512, 256, 3var, 3dx]
    fcw = np.ascontiguousarray(
        wsum.reshape(4, 128, 2, 128, 3, 3).transpose(3, 2, 4, 5, 0, 1))
    fb4 = np.ascontiguousarray(
        np.asarray(inputs["fb"], f).reshape(4, 128).T)

    wc1 = np.asarray(inputs["wc1"], f).reshape(64, 11)
    w1w = np.zeros((5, 3, 64), f)
    for tap in range(11):
        w1w[tap % 5, tap // 5, :] = wc1[:, tap]
    w2w = np.ascontiguousarray(
        np.asarray(inputs["wc2"], f).reshape(128, 64, 5)
        .transpose(1, 2, 0))
    w3w = np.ascontiguousarray(
        np.asarray(inputs["wc3"], f).reshape(256, 128, 5)
        .transpose(1, 2, 0).reshape(128, 5, 2, 128))
    wb3w = np.ascontiguousarray(
        np.asarray(inputs["wb3"], f).reshape(2, 128).T)
    w4wT = np.ascontiguousarray(
        np.asarray(inputs["wc4"], f).reshape(256, 256, 5)
        .transpose(1, 2, 0).reshape(2, 128, 5, 256).transpose(1, 0, 2, 3))
    wb4row = np.asarray(inputs["wb4"], f).reshape(1, 256)
    one50 = np.ones((1, 50), f)

    def gru_prep(d):
        wih = np.asarray(inputs[f"wih_{d}"], f) / 512.0
        whh = np.asarray(inputs[f"whh_{d}"], f)
        bih = np.asarray(inputs[f"bih_{d}"], f)
        bhh = np.asarray(inputs[f"bhh_{d}"], f)
        wihT = np.ascontiguousarray(
            wih.reshape(3, 128, 4, 128).transpose(3, 0, 2, 1))
        whhT = np.ascontiguousarray(
            whh.reshape(3, 128, 128).transpose(2, 0, 1))
        brz = (bih + bhh)[:256].reshape(2, 128).T
        return wihT, whhT, brz, bih[256:], bhh[256:]

    wihT_f, whhT_f, brz_f, bn_f, bhn_f = gru_prep("f")
    wihT_b, whhT_b, brz_b, bn_b, bhn_b = gru_prep("b")
    wihT = np.ascontiguousarray(np.stack([wihT_f, wihT_b], axis=1))
    whhT = np.ascontiguousarray(np.stack([whhT_f, whhT_b], axis=1))
    brz = np.ascontiguousarray(np.stack([brz_f, brz_b], axis=1))
    brz[:, :, 1] *= -1.0
    bng = np.ascontiguousarray(np.stack([bn_f, bn_b], axis=1))
    bhhn = np.ascontiguousarray(np.stack([bhn_f, bhn_b], axis=1))
    clsW = np.asarray(inputs["clsW"], f) / 32.0
    clsT = np.ascontiguousarray(
        clsW.reshape(5, 2, 128).transpose(2, 1, 0))
    clsb5 = np.asarray(inputs["clsb"], f).reshape(5, 1)

    shared = dict(
        wP=wP, w1m=w1m, bn1s=s1.reshape(64, 1), bn1b=b1.reshape(64, 1),
        w2m=w2m, bn2s=s2.reshape(128, 1), bn2b=b2.reshape(128, 1),
        w3m=w3m, b3=b3, fcm=fcm, fcw=fcw, fb4=fb4,
        w1w=w1w, wb1=np.asarray(inputs["wb1"], f).reshape(64, 1),
        w2w=w2w, wb2=np.asarray(inputs["wb2"], f).reshape(128, 1),
        w3w=w3w, wb3w=wb3w, w4wT=w4wT, wb4row=wb4row, one50=one50,
        wihT=wihT, brz=brz, bng=bng, bhhn=bhhn, whhT=whhT,
        clsT=clsT, clsb5=clsb5, zed=np.zeros((128, 2), f),
    )
    in_maps = []
    for i in range(NCORES):
        m = dict(shared)
        lms = lmp[:, 128 * i:128 * i + 136]
        lmI = np.empty((9, 8576), f)
        for dy in range(3):
            for dx in range(3):
                lmI[3 * dy + dx] = lms[dy:dy + 64, dx:dx + 134].reshape(-1)
        m["lmI"] = lmI
        m["Ri"] = np.ascontiguousarray(Rp[:, 64 * i:64 * i + 66])
        mk = np.ones((128, 2), f)
        if i == 0:
            mk[:, 0] = 0.0
        if i == NCORES - 1:
            mk[:, 1] = 0.0
        m["msk"] = mk
        in_maps.append(m)
    return in_maps


def kernel(**inputs):
    global LAST_RESULTS
    _ensure_concourse()
    from concourse import bass_utils

    if "nc" not in _CACHE:
        _CACHE["nc"] = _build()
    nc = _CACHE["nc"]
    in_maps = _prep_inputs(inputs)
    res = bass_utils.run_bass_kernel_spmd(
        nc, in_maps, core_ids=list(range(NCORES)))
    LAST_RESULTS = res
    return res.results[0]["out"]


if __name__ == "__main__":
    _ensure_concourse()
    _build()
    print("build + compile OK")



# revision 19
# speedup vs baseline: 1.3859x; 1.3859x over previous
"""Trainium2 Bass kernel for nn_CNNGRUforHorizon (CNN+BiGRU audio model).

Strategy: W-shard the logmel branch + fusion conv across 8 cores (each core
owns 64 of the 512 fused-map columns, with halo), replicate the tiny wave
branch, AllReduce the feature matrix per oc-chunk (overlapped with the
fusion conv), then run the 32-step BiGRU replicated on every core.

v2: all matmuls in bf16 (fp32 LDWEIGHTS at 4 cyc/row was co-critical with
MATMUL in the fusion conv). The wave half of the fusion conv is folded into
per-(oc,w) constants cw (x_wave rows are identical across H), applied via
the identity sum_w relu(x+c) = sum_w max(x,-c) + sum_w c, so the fc conv
only runs mel-half matmuls. m1's row-shifted copy uses partition-shifted
ACT writes from PSUM instead of SBUF-SBUF DMA.
"""
import os
import sys

import numpy as np


def _ensure_concourse():
    try:
        import concourse  # noqa: F401
        return
    except ImportError:
        pass
    for p in ("/opt/trn_rl_repo", "/root/.axon_site/_ro/trn_rl_repo"):
        if os.path.isdir(p) and p not in sys.path:
            sys.path.insert(0, p)
    import concourse  # noqa: F401


NCORES = 8
LAST_RESULTS = None
_CACHE = {}
DEBUG_TAPS = False


def _resize_matrix(n_in, n_out):
    R = np.zeros((n_in, n_out), np.float64)
    for x in range(n_out):
        c = (x + 0.5) * n_in / n_out - 0.5
        i0 = int(np.floor(c))
        w1 = c - i0
        i0c = min(max(i0, 0), n_in - 1)
        i1c = min(max(i0 + 1, 0), n_in - 1)
        R[i0c, x] += 1.0 - w1
        R[i1c, x] += w1
    return R.astype(np.float32)


def _build():
    import concourse.bacc as bacc
    import concourse.mybir as mybir
    import concourse.tile as tile

    f32 = mybir.dt.float32
    bf16 = mybir.dt.bfloat16
    AF = mybir.ActivationFunctionType
    ALU = mybir.AluOpType
    AX = mybir.AxisListType

    nc = bacc.Bacc("TRN2", target_bir_lowering=False, debug=False,
                   num_devices=NCORES)

    def din(name, shape, dt=bf16):
        return nc.dram_tensor(name, shape, dt, kind="ExternalInput")

    lmI_d = din("lmI", [9, 8576])            # logmel im2col slice, per core
    wP_d = din("wP", [5, 3203])              # wave, stride-5 im2col planes
    Ri_d = din("Ri", [50, 66])               # resize matrix slice, per core
    msk_d = din("msk", [128, 2], f32)        # x_mel edge-col masks, per core
    zed_d = din("zed", [128, 2], f32)        # zeros (warmup CC input)

    w1m_d = din("w1m", [9, 64])
    bn1s_d = din("bn1s", [64, 1], f32)
    bn1b_d = din("bn1b", [64, 1], f32)
    w2m_d = din("w2m", [128, 6, 128])
    bn2s_d = din("bn2s", [128, 1], f32)
    bn2b_d = din("bn2b", [128, 1], f32)
    w3m_d = din("w3m", [128, 2, 128])
    b3_d = din("b3", [128, 2], f32)
    fcm_d = din("fcm", [128, 2, 9, 4, 128])      # fc mel-half weights
    fcwd_d = din("fcwd", [128, 2, 3, 3, 4, 128])  # fc wave-half, per-dy, negated
    nfb4_d = din("nfb4", [128, 4], f32)          # -fb

    w1w_d = din("w1w", [5, 3, 64])
    wb1_d = din("wb1", [64, 1], f32)
    w2w_d = din("w2w", [64, 5, 128])
    wb2_d = din("wb2", [128, 1], f32)
    w3w_d = din("w3w", [128, 5, 2, 128])
    wb3w_d = din("wb3w", [128, 2], f32)
    w4wT_d = din("w4wT", [128, 2, 5, 256])
    wb4row_d = din("wb4row", [1, 256])
    one50_d = din("one50", [1, 50])

    wihT_d = din("wihT", [128, 2, 3, 4, 128])
    brz_d = din("brz", [128, 2, 2], f32)
    bng_d = din("bng", [128, 2], f32)
    bhhn_d = din("bhhn", [128, 2], f32)
    whhT_d = din("whhT", [128, 2, 3, 128])
    clsT_d = din("clsT", [128, 2, 5])
    clsb5_d = din("clsb5", [5, 1], f32)

    out_d = nc.dram_tensor("out", [1, 5], f32, kind="ExternalOutput")
    if DEBUG_TAPS:
        dbg_xw1_d = nc.dram_tensor("dbg_xw1", [128, 2, 66], bf16,
                                   kind="ExternalOutput")
        dbg_xmel_d = nc.dram_tensor("dbg_xmel", [128, 2, 34, 66], bf16,
                                    kind="ExternalOutput")
        dbg_featp_d = nc.dram_tensor("dbg_featp", [128, 4, 32], f32,
                                     kind="ExternalOutput")
        dbg_negC_d = nc.dram_tensor("dbg_negC", [128, 4, 3, 64], f32,
                                    kind="ExternalOutput")
        dbg_ftr_d = nc.dram_tensor("dbg_ftr", [128, 4, 32], bf16,
                                   kind="ExternalOutput")
        dbg_ggxn_d = nc.dram_tensor("dbg_ggxn", [128, 2, 32], f32,
                                    kind="ExternalOutput")
        dbg_ggxrz_d = nc.dram_tensor("dbg_ggxrz", [128, 2, 32, 2], f32,
                                     kind="ExternalOutput")
        dbg_hall_d = nc.dram_tensor("dbg_hall", [128, 2, 33, 2], bf16,
                                    kind="ExternalOutput")
        dbg_pgx_d = nc.dram_tensor("dbg_pgx", [128, 6, 32], f32,
                                   kind="ExternalOutput")

    with tile.TileContext(nc) as tc:
        with (
            tc.tile_pool(name="keep", bufs=1) as keep,      # long-lived
            tc.tile_pool(name="psum", bufs=1, space="PSUM") as psp,
            tc.tile_pool(name="dram", bufs=1, space="DRAM") as dram,
            tc.tile_pool(name="sc", bufs=1) as sc,          # small scratch
        ):
            # ---- long-lived tiles + weight DMAs ----
            msk = keep.tile([128, 2], f32)
            nc.sync.dma_start(msk[:], msk_d[:])
            xmel = keep.tile([128, 2, 34, 66], bf16)
            featp = keep.tile([128, 4, 32], f32)
            xw1 = keep.tile([128, 2, 66], bf16)
            ncd = keep.tile([128, 4, 3, 64], f32)   # -c_dy per oc chunk
            negC = keep.tile([128, 4, 3, 64], f32)  # -(c_var + b)
            negsums = keep.tile([128, 4, 3], f32)
            nfb4 = keep.tile([128, 4], f32)
            nc.sync.dma_start(nfb4[:], nfb4_d[:])
            fcm = keep.tile([128, 2, 9, 4, 128], bf16)
            nc.gpsimd.dma_start(fcm[:], fcm_d[:])
            fcwd = keep.tile([128, 2, 3, 3, 4, 128], bf16)
            nc.gpsimd.dma_start(fcwd[:], fcwd_d[:])

            def psum_mm(shape):
                return psp.tile(shape, f32, tag="mm", bufs=3, name="psmm")

            def psum_aux(shape):
                return psp.tile(shape, f32, tag="aux", bufs=3, name="psaux")

            # warmup collective early (absorbs CC engine startup)
            ccz_i = dram.tile([128, 2], f32)
            ccz_o = dram.tile([128, 2], f32)
            nc.sync.dma_start(ccz_i[:], zed_d[:])
            nc.gpsimd.collective_compute(
                "AllReduce", ALU.add,
                replica_groups=[list(range(NCORES))],
                ins=[ccz_i.opt()], outs=[ccz_o.opt()])

            # mel m1 pools (allocated first so wave pool releases first)
            p_m1 = tc.alloc_tile_pool(name="mel_m1", bufs=1)
            p_rhs = tc.alloc_tile_pool(name="mel_rhs", bufs=1)
            # m1 flat [128, 8848]: partitions 0-63 hold row slot r at
            # offset 134*r (slots 0..65); partitions 64-127 hold the
            # same data shifted one row (for K=128 dy-packed m2 matmuls)
            m1 = p_m1.tile([128, 8848], bf16, tag="m1")
            rhs1 = p_rhs.tile([9, 8576], bf16)
            nc.sync.dma_start(rhs1[:], lmI_d[:])
            w1m = keep.tile([9, 64], bf16)
            nc.sync.dma_start(w1m[:], w1m_d[:])
            bn1s = keep.tile([64, 1], f32)
            nc.sync.dma_start(bn1s[:], bn1s_d[:])
            bn1b = keep.tile([64, 1], f32)
            nc.sync.dma_start(bn1b[:], bn1b_d[:])
            w2m = keep.tile([128, 6, 128], bf16)
            nc.sync.dma_start(w2m[:], w2m_d[:])
            bn2s = keep.tile([128, 1], f32)
            nc.sync.dma_start(bn2s[:], bn2s_d[:])
            bn2b = keep.tile([128, 1], f32)
            nc.sync.dma_start(bn2b[:], bn2b_d[:])
            w3m = keep.tile([128, 2, 128], bf16)
            nc.sync.dma_start(w3m[:], w3m_d[:])
            b3 = keep.tile([128, 2], f32)
            nc.sync.dma_start(b3[:], b3_d[:])

            nc.vector.memset(m1[0:64, 0:134], 0.0)
            nc.vector.memset(m1[0:64, 8710:8848], 0.0)
            nc.vector.memset(m1[64:128, 8576:8848], 0.0)

            def m1_chunk(c):
                n0 = 512 * c
                n = min(512, 8576 - n0)
                ps = psum_mm([64, 512])
                nc.tensor.matmul(ps[:, :n], w1m[:], rhs1[:, n0:n0 + n],
                                 start=True, stop=True)
                nc.scalar.activation(m1[0:64, 134 + n0:134 + n0 + n],
                                     ps[:, :n], AF.Relu,
                                     bias=bn1b[:, 0:1], scale=bn1s[:, 0:1])
                # partition-shifted second write: upper half = lower
                # shifted one row-slot (upper[p, j] = lower[p-64, j+134])
                nc.scalar.activation(m1[64:128, n0:n0 + n],
                                     ps[:, :n], AF.Relu,
                                     bias=bn1b[:, 0:1], scale=bn1s[:, 0:1])

            # ============== WAVE BRANCH (replicated), interleaved with m1
            with tc.tile_pool(name="wave", bufs=1) as wv:
                Pt = wv.tile([5, 3203], bf16)
                nc.sync.dma_start(Pt[:], wP_d[:])
                w1w = wv.tile([5, 3, 64], bf16)
                nc.sync.dma_start(w1w[:], w1w_d[:])
                wb1 = wv.tile([64, 1], f32)
                nc.sync.dma_start(wb1[:], wb1_d[:])
                w2w = wv.tile([64, 5, 128], bf16)
                nc.sync.dma_start(w2w[:], w2w_d[:])
                wb2 = wv.tile([128, 1], f32)
                nc.sync.dma_start(wb2[:], wb2_d[:])
                w3w = wv.tile([128, 5, 2, 128], bf16)
                nc.sync.dma_start(w3w[:], w3w_d[:])
                wb3w = wv.tile([128, 2], f32)
                nc.sync.dma_start(wb3w[:], wb3w_d[:])
                w4wT = wv.tile([128, 2, 5, 256], bf16)
                nc.sync.dma_start(w4wT[:], w4wT_d[:])
                wb4row = wv.tile([1, 256], bf16)
                nc.sync.dma_start(wb4row[:], wb4row_d[:])
                one50 = wv.tile([1, 50], bf16)
                nc.sync.dma_start(one50[:], one50_d[:])
                Ri = wv.tile([50, 66], bf16)
                nc.sync.dma_start(Ri[:], Ri_d[:])

                # conv1: 16000 -> 3200, k=11 s=5 via 3 taps of K=5
                w1o = wv.tile([64, 3200], bf16)
                for c in range(7):
                    n0 = 512 * c
                    n = min(512, 3200 - n0)
                    ps = psum_aux([64, 512])
                    for m in range(3):
                        nc.tensor.matmul(ps[:, :n], w1w[:, m, :],
                                         Pt[:, m + n0:m + n0 + n],
                                         start=(m == 0), stop=(m == 2))
                    nc.scalar.activation(w1o[:, n0:n0 + n], ps[:, :n],
                                         AF.Relu, bias=wb1[:, 0:1])

                for c in range(17):
                    m1_chunk(c)

                p1t = wv.tile([64, 804], bf16)
                nc.vector.memset(p1t[:, 0:2], 0.0)
                nc.vector.memset(p1t[:, 802:804], 0.0)
                t1 = wv.tile([64, 800], bf16)
                t2 = wv.tile([64, 800], bf16)
                nc.vector.tensor_tensor(t1[:], w1o[:, 0:3200:4],
                                        w1o[:, 1:3200:4], op=ALU.max)
                nc.vector.tensor_tensor(t2[:], w1o[:, 2:3200:4],
                                        w1o[:, 3:3200:4], op=ALU.max)
                nc.vector.tensor_tensor(p1t[:, 2:802], t1[:], t2[:],
                                        op=ALU.max)
                # conv2: k=5 pad 2, 64 -> 128 ch, 800 cols
                w2o = wv.tile([128, 800], bf16)
                for c in range(2):
                    n0 = 512 * c
                    n = min(512, 800 - n0)
                    ps = psum_aux([128, 512])
                    for tap in range(5):
                        nc.tensor.matmul(ps[:, :n], w2w[:, tap, :],
                                         p1t[:, n0 + tap:n0 + tap + n],
                                         start=(tap == 0), stop=(tap == 4))
                    nc.scalar.activation(w2o[:, n0:n0 + n], ps[:, :n],
                                         AF.Relu, bias=wb2[:, 0:1])
                p2t = wv.tile([128, 204], bf16)
                nc.vector.memset(p2t[:, 0:2], 0.0)
                nc.vector.memset(p2t[:, 202:204], 0.0)
                t3 = wv.tile([128, 200], bf16)
                t4 = wv.tile([128, 200], bf16)
                nc.vector.tensor_tensor(t3[:], w2o[:, 0:800:4],
                                        w2o[:, 1:800:4], op=ALU.max)
                nc.vector.tensor_tensor(t4[:], w2o[:, 2:800:4],
                                        w2o[:, 3:800:4], op=ALU.max)
                nc.vector.tensor_tensor(p2t[:, 2:202], t3[:], t4[:],
                                        op=ALU.max)
                # conv3: k=5 pad 2, 128 -> 256 ch, 200 cols
                w3o = wv.tile([128, 2, 200], bf16)
                for oc in range(2):
                    ps = psum_aux([128, 512])
                    for tap in range(5):
                        nc.tensor.matmul(ps[:, :200], w3w[:, tap, oc, :],
                                         p2t[:, tap:tap + 200],
                                         start=(tap == 0), stop=(tap == 4))
                    nc.scalar.activation(w3o[:, oc, :], ps[:, :200],
                                         AF.Relu, bias=wb3w[:, oc:oc + 1])
                p3t = wv.tile([128, 2, 54], bf16)
                for oc in range(2):
                    nc.vector.memset(p3t[:, oc, 0:2], 0.0)
                    nc.vector.memset(p3t[:, oc, 52:54], 0.0)
                    t5 = wv.tile([128, 50], bf16, tag="t5")
                    t6 = wv.tile([128, 50], bf16, tag="t6")
                    nc.vector.tensor_tensor(t5[:], w3o[:, oc, 0:200:4],
                                            w3o[:, oc, 1:200:4], op=ALU.max)
                    nc.vector.tensor_tensor(t6[:], w3o[:, oc, 2:200:4],
                                            w3o[:, oc, 3:200:4], op=ALU.max)
                    nc.vector.tensor_tensor(p3t[:, oc, 2:52], t5[:], t6[:],
                                            op=ALU.max)
                # conv4 (transposed out): k=5 pad 2, 256 -> 256 ch, 50 cols
                ps4 = psum_aux([50, 256])
                first = True
                for ch in range(2):
                    for tap in range(5):
                        nc.tensor.matmul(ps4[:], p3t[:, ch, tap:tap + 50],
                                         w4wT[:, ch, tap, :],
                                         start=first, stop=False)
                        first = False
                nc.tensor.matmul(ps4[:], one50[:], wb4row[:],
                                 start=False, stop=True)
                xwT = wv.tile([50, 256], bf16)
                nc.scalar.activation(xwT[:], ps4[:], AF.Relu)
                # resize 50 -> local 66 cols (per-core R slice)
                for oc in range(2):
                    psR = psum_aux([128, 66])
                    nc.tensor.matmul(psR[:], xwT[:, 128 * oc:128 * (oc + 1)],
                                     Ri[:], start=True, stop=True)
                    nc.vector.tensor_copy(xw1[:, oc, :], psR[:])

            # ============== MEL m2 / m3 (W-sharded) ======================
            for oc in range(2):
                nc.vector.memset(xmel[:, oc, 0, :], 0.0)
                nc.vector.memset(xmel[:, oc, 33, :], 0.0)

            p_rhs.release()
            # m2: 3x3 conv, 64 -> 128 ch, flat 64x134 grid (2 junk cols)
            p_m2 = tc.alloc_tile_pool(name="mel_m2", bufs=1)
            m2 = p_m2.tile([128, 8576], bf16)
            for c in range(17):
                n0 = 512 * c
                n = min(512, 8576 - n0)
                ps = psum_mm([128, 512])
                for j in range(6):
                    off = (268 if j >= 3 else 0) + (j % 3) + n0
                    nc.tensor.matmul(
                        ps[:, :n], w2m[:, j, :],
                        m1[:, off:off + n],
                        start=(j == 0), stop=(j == 5))
                nc.scalar.activation(m2[:, n0:n0 + n],
                                     ps[:, :n], AF.Relu,
                                     bias=bn2b[:, 0:1],
                                     scale=bn2s[:, 0:1])

            # m3 (1x1 conv, 128 -> 256) + maxpool 2x2 per chunk
            p_m3 = tc.alloc_tile_pool(name="mel_m3", bufs=1)
            for oc in range(2):
                m3 = p_m3.tile([128, 8576], bf16, tag="m3",
                               bufs=2, name="m3")
                for c in range(17):
                    n0 = 512 * c
                    n = min(512, 8576 - n0)
                    ps = psum_mm([128, 512])
                    nc.tensor.matmul(ps[:, :n], w3m[:, oc, :],
                                     m2[:, n0:n0 + n],
                                     start=True, stop=True)
                    if oc == 0:
                        nc.scalar.activation(
                            m3[:, n0:n0 + n], ps[:, :n], AF.Relu,
                            bias=b3[:, oc:oc + 1])
                    else:
                        nc.vector.tensor_scalar(
                            m3[:, n0:n0 + n], ps[:, :n],
                            b3[:, oc:oc + 1], 0.0,
                            op0=ALU.add, op1=ALU.max)
                m3v = m3.rearrange("p (a b) -> p a b", b=134)
                vp = p_m3.tile([128, 32, 134], bf16, tag="vp",
                               bufs=1, name="vp")
                nc.vector.tensor_tensor(
                    vp[:], m3v[:, 0:64:2, :], m3v[:, 1:64:2, :],
                    op=ALU.max)
                nc.vector.tensor_tensor(
                    xmel[:, oc, 1:33, :], vp[:, :, 0:132:2],
                    vp[:, :, 1:132:2], op=ALU.max)
            for j, col in ((0, 0), (1, 65)):
                nc.vector.tensor_scalar_mul(
                    xmel[:, :, 1:33, col:col + 1],
                    xmel[:, :, 1:33, col:col + 1],
                    msk[:, j:j + 1])
            p_m3.release()
            p_m2.release()
            p_m1.release()

            # ============== cw: wave-half fc constants ==================
            # ncd[:, oc, dy, :] = -c_dy (fcwd is negated on host)
            for oc in range(4):
                for dy in range(3):
                    pcw = psum_aux([128, 64])
                    first = True
                    for ch in range(2):
                        for dx in range(3):
                            nc.tensor.matmul(
                                pcw[:], fcwd[:, ch, dy, dx, oc, :],
                                xw1[:, ch, dx:dx + 64],
                                start=first,
                                stop=(ch == 1 and dx == 2))
                            first = False
                    nc.scalar.activation(ncd[:, oc, dy, :], pcw[:],
                                         AF.Identity)
            # negC variants: int = -(c0+c1+c2+b), top = -(c1+c2+b),
            # bot = -(c0+c1+b)
            for oc in range(4):
                nc.vector.scalar_tensor_tensor(
                    negC[:, oc, 1, :], ncd[:, oc, 1, :],
                    nfb4[:, oc:oc + 1], ncd[:, oc, 2, :],
                    op0=ALU.add, op1=ALU.add)
                nc.vector.tensor_tensor(
                    negC[:, oc, 0, :], negC[:, oc, 1, :], ncd[:, oc, 0, :],
                    op=ALU.add)
                nc.vector.scalar_tensor_tensor(
                    negC[:, oc, 2, :], ncd[:, oc, 1, :],
                    nfb4[:, oc:oc + 1], ncd[:, oc, 0, :],
                    op0=ALU.add, op1=ALU.add)
            nc.vector.tensor_reduce(negsums[:], negC[:],
                                    axis=AX.X, op=ALU.add)

            # GRU weights/buffers pool (small, lives to the end)
            p_gru = tc.alloc_tile_pool(name="gru", bufs=1)
            wihT = p_gru.tile([128, 2, 3, 4, 128], bf16)
            nc.sync.dma_start(wihT[:], wihT_d[:])
            brz = p_gru.tile([128, 2, 2], f32)
            nc.sync.dma_start(brz[:], brz_d[:])
            bng = p_gru.tile([128, 2], f32)
            nc.sync.dma_start(bng[:], bng_d[:])
            bhhn = p_gru.tile([128, 2], f32)
            nc.sync.dma_start(bhhn[:], bhhn_d[:])
            whhT = p_gru.tile([128, 2, 3, 128], bf16)
            nc.sync.dma_start(whhT[:], whhT_d[:])
            clsT = p_gru.tile([128, 2, 5], bf16)
            nc.sync.dma_start(clsT[:], clsT_d[:])
            clsb5 = p_gru.tile([5, 1], f32)
            nc.sync.dma_start(clsb5[:], clsb5_d[:])
            ggxrz = p_gru.tile([128, 2, 32, 2], f32)
            ggxn = p_gru.tile([128, 2, 32], f32)
            ft = p_gru.tile([128, 4, 32], f32)
            ftr = p_gru.tile([128, 4, 32], bf16)
            # hall[:, d, s, 0]: hidden state per step (col 1 stays zero)
            hall = p_gru.tile([128, 2, 33, 2], bf16)
            nc.vector.memset(hall[:], 0.0)
            hsum = p_gru.tile([128, 2, 2], bf16)
            nc.vector.memset(hsum[:], 0.0)
            # gx accumulator in SBUF (open PSUM groups must not interleave
            # within a bank, so each kk chunk is a self-contained matmul)
            pgx = p_gru.tile([128, 6, 32], f32, name="pgx")

            # ============== FC FUSION CONV + per-oc AllReduce ===========
            ccin = [dram.tile([128, 32], f32, tag=f"ci{i}", name=f"ccin{i}")
                    for i in range(4)]
            ccout = [dram.tile([128, 32], f32, tag=f"co{i}", name=f"ccout{i}")
                     for i in range(4)]

            def fc_oc(oc):
                for rg in range(4):
                    ps = psum_mm([128, 8, 64])
                    first = True
                    for ch in range(2):
                        for dy in range(3):
                            for dx in range(3):
                                nc.tensor.matmul(
                                    ps[:],
                                    fcm[:, ch, 3 * dy + dx, oc, :],
                                    xmel[:, ch, rg * 8 + dy:
                                         rg * 8 + dy + 8, dx:dx + 64],
                                    start=first,
                                    stop=(ch == 1 and dy == 2 and dx == 2))
                                first = False
                    # relu via max(x, -(cw+b)); constants re-added after
                    xft = sc.tile([128, 8, 64], f32, tag="xf", bufs=3)
                    if rg == 0:
                        nc.vector.tensor_tensor(
                            xft[:, 0:1, :], ps[:, 0:1, :],
                            negC[:, oc, 1:2, :], op=ALU.max)
                        nc.vector.tensor_tensor(
                            xft[:, 1:8, :], ps[:, 1:8, :],
                            negC[:, oc, 0, :][:, None, :]
                            .broadcast_to([128, 7, 64]), op=ALU.max)
                    elif rg == 3:
                        nc.vector.tensor_tensor(
                            xft[:, 0:7, :], ps[:, 0:7, :],
                            negC[:, oc, 0, :][:, None, :]
                            .broadcast_to([128, 7, 64]), op=ALU.max)
                        nc.vector.tensor_tensor(
                            xft[:, 7:8, :], ps[:, 7:8, :],
                            negC[:, oc, 2:3, :], op=ALU.max)
                    else:
                        nc.vector.tensor_tensor(
                            xft[:], ps[:],
                            negC[:, oc, 0, :][:, None, :]
                            .broadcast_to([128, 8, 64]), op=ALU.max)
                    nc.vector.tensor_reduce(
                        featp[:, oc, rg * 8:rg * 8 + 8], xft[:],
                        axis=AX.X, op=ALU.add)
                # re-add sum_w (cw + b) per variant
                nc.vector.tensor_tensor(
                    featp[:, oc, :], featp[:, oc, :],
                    negsums[:, oc, 0:1].broadcast_to([128, 32]),
                    op=ALU.subtract)
                nc.vector.scalar_tensor_tensor(
                    featp[:, oc, 0:1], featp[:, oc, 0:1],
                    negsums[:, oc, 1:2], negsums[:, oc, 0:1],
                    op0=ALU.subtract, op1=ALU.add)
                nc.vector.scalar_tensor_tensor(
                    featp[:, oc, 31:32], featp[:, oc, 31:32],
                    negsums[:, oc, 2:3], negsums[:, oc, 0:1],
                    op0=ALU.subtract, op1=ALU.add)
                # allreduce this oc chunk
                nc.sync.dma_start(ccin[oc][:], featp[:, oc, :])
                nc.gpsimd.collective_compute(
                    "AllReduce", ALU.add,
                    replica_groups=[list(range(NCORES))],
                    ins=[ccin[oc].opt()], outs=[ccout[oc].opt()])
                nc.sync.dma_start(ft[:, oc, :], ccout[oc][:])
                nc.vector.tensor_copy(ftr[:, oc, :], ft[:, oc, :])

            def gx_chunk(kk):
                psk = psp.tile([128, 6, 32], f32, tag="gx", bufs=2,
                               name="psk")
                for d in range(2):
                    for g in range(3):
                        nc.tensor.matmul(psk[:, 3 * d + g, :],
                                         wihT[:, d, g, kk, :],
                                         ftr[:, kk, :],
                                         start=True, stop=True)
                if kk == 0:
                    nc.vector.tensor_copy(pgx[:], psk[:])
                else:
                    nc.vector.tensor_tensor(pgx[:], pgx[:], psk[:],
                                            op=ALU.add)

            fc_oc(0)
            fc_oc(1)
            gx_chunk(0)
            fc_oc(2)
            gx_chunk(1)
            fc_oc(3)
            gx_chunk(2)
            gx_chunk(3)

            if DEBUG_TAPS:
                nc.sync.dma_start(dbg_xw1_d[:], xw1[:])
                nc.sync.dma_start(dbg_xmel_d[:], xmel[:])
                nc.sync.dma_start(dbg_featp_d[:], featp[:])
                nc.sync.dma_start(dbg_negC_d[:], negC[:])
                nc.sync.dma_start(dbg_ftr_d[:], ftr[:])

            # gate biases: ggxrz (r, z with sign flip), ggxn
            for d in range(2):
                nc.scalar.activation(ggxrz[:, d, :, 0], pgx[:, 3 * d + 0, :],
                                     AF.Identity, bias=brz[:, d, 0:1])
                nc.scalar.activation(ggxrz[:, d, :, 1], pgx[:, 3 * d + 1, :],
                                     AF.Identity, bias=brz[:, d, 1:2],
                                     scale=-1.0)
                nc.scalar.activation(ggxn[:, d, :], pgx[:, 3 * d + 2, :],
                                     AF.Identity, bias=bng[:, d:d + 1])

            # ============== GRU (replicated) =============================
            for s in range(32):
                for d in range(2):
                    t = s if d == 0 else 31 - s
                    ps = psum_aux([128, 3, 2])
                    for g in (0, 2, 1):  # r first (unblocks sigmoid), z last
                        nc.tensor.matmul(ps[:, g, :], whhT[:, d, g, :],
                                         hall[:, d, s, :],
                                         start=True, stop=True)
                    rz = sc.tile([128, 2], f32, tag="rz", bufs=4)
                    nc.scalar.activation(rz[:, 0:1], ps[:, 0, 0:1],
                                         AF.Sigmoid,
                                         bias=ggxrz[:, d, t, 0:1])
                    nc.scalar.activation(rz[:, 1:2], ps[:, 1, 0:1],
                                         AF.Sigmoid, scale=-1.0,
                                         bias=ggxrz[:, d, t, 1:2])
                    tn = sc.tile([128, 1], f32, tag="tn", bufs=4)
                    nc.vector.scalar_tensor_tensor(
                        tn[:], ps[:, 2, 0:1], bhhn[:, d:d + 1], rz[:, 0:1],
                        op0=ALU.add, op1=ALU.mult)
                    nt = sc.tile([128, 1], f32, tag="nt", bufs=4)
                    nc.scalar.activation(nt[:], tn[:], AF.Tanh,
                                         bias=ggxn[:, d, t:t + 1])
                    # rz[:,1] holds z' = 1-z.  t1 = h*z' - h (off critical
                    # path); h' = n*z' - t1 = (1-z)*n + z*h
                    hmn = sc.tile([128, 1], f32, tag="hmn", bufs=4)
                    nc.vector.scalar_tensor_tensor(
                        hmn[:], hall[:, d, s, 0:1], rz[:, 1:2],
                        hall[:, d, s, 0:1], op0=ALU.mult, op1=ALU.subtract)
                    nc.vector.scalar_tensor_tensor(
                        hall[:, d, s + 1, 0:1], nt[:], rz[:, 1:2], hmn[:],
                        op0=ALU.mult, op1=ALU.subtract)

            if DEBUG_TAPS:
                nc.sync.dma_start(dbg_pgx_d[:], pgx[:])
                nc.sync.dma_start(dbg_ggxn_d[:], ggxn[:])
                nc.sync.dma_start(dbg_ggxrz_d[:], ggxrz[:])
                nc.sync.dma_start(dbg_hall_d[:], hall[:])
            with nc.allow_low_precision(reason="hsum mean of 32 gated states"):
                nc.vector.tensor_reduce(hsum[:, :, 0], hall[:, :, 1:33, 0],
                                        axis=AX.X, op=ALU.add)
            psc = psum_aux([5, 2])
            for d in range(2):
                nc.tensor.matmul(psc[:], clsT[:, d, :], hsum[:, d, :],
                                 start=(d == 0), stop=(d == 1))
            lgt = sc.tile([5, 1], f32, tag="lgt")
            nc.scalar.activation(lgt[:], psc[:, 0:1], AF.Identity,
                                 bias=clsb5[:, 0:1])
            nc.sync.dma_start(out_d[0:1, :].rearrange("a p -> p a"), lgt[:])
            p_gru.release()

    nc.compile()
    return nc


def _prep_inputs(inputs):
    """Build the 8 per-core input maps from the full model inputs."""
    import ml_dtypes
    f = np.float32
    bf = ml_dtypes.bfloat16

    def b(x):
        return np.ascontiguousarray(np.asarray(x, f)).astype(bf)

    wave = np.asarray(inputs["waveform"], f).reshape(16000)
    logmel = np.asarray(inputs["logmel"], f).reshape(64, 1024)

    wp = np.zeros(16015, f)
    wp[3:16003] = wave
    wP = b(wp.reshape(3203, 5).T)                      # [5, 3203]

    R = _resize_matrix(50, 512)
    Rp = np.zeros((50, 514), f)
    Rp[:, 1:513] = R

    lmp = np.pad(logmel, ((1, 1), (4, 4)))

    w1m = b(np.asarray(inputs["mc1"], f).reshape(64, 9).T)
    s1 = np.asarray(inputs["bn1g"], f) / np.sqrt(
        np.asarray(inputs["bn1v"], f) + 1e-5)
    b1 = (np.asarray(inputs["mb1"], f) - np.asarray(inputs["bn1m"], f)) * s1 \
        + np.asarray(inputs["bn1b"], f)
    mc2 = np.asarray(inputs["mc2"], f)              # [128, 64, 3, 3]
    w2m = np.zeros((128, 6, 128), f)
    for dx in range(3):
        w2m[0:64, dx, :] = mc2[:, :, 0, dx].T
        w2m[64:128, dx, :] = mc2[:, :, 1, dx].T
        w2m[0:64, 3 + dx, :] = mc2[:, :, 2, dx].T
    s2 = np.asarray(inputs["bn2g"], f) / np.sqrt(
        np.asarray(inputs["bn2v"], f) + 1e-5)
    b2 = (np.asarray(inputs["mb2"], f) - np.asarray(inputs["bn2m"], f)) * s2 \
        + np.asarray(inputs["bn2b"], f)
    w3m = b(np.asarray(inputs["mc3"], f).reshape(256, 128).T
            .reshape(128, 2, 128))
    b3 = np.ascontiguousarray(
        np.asarray(inputs["mb3"], f).reshape(2, 128).T)

    fc = np.asarray(inputs["fc"], f)                   # [512,512,3,3]
    fcmel = fc[:, 256:, :, :]
    fcm = b(fcmel.reshape(4, 128, 2, 128, 9).transpose(3, 2, 4, 0, 1))
    fcwave = fc[:, :256, :, :]                         # [512, 256, 3, 3]
    fcwd = b((-fcwave).reshape(4, 128, 2, 128, 3, 3)
             .transpose(3, 2, 4, 5, 0, 1))             # [128,2,3dy,3dx,4,128]
    nfb4 = np.ascontiguousarray(
        -np.asarray(inputs["fb"], f).reshape(4, 128).T)

    wc1 = np.asarray(inputs["wc1"], f).reshape(64, 11)
    w1w = np.zeros((5, 3, 64), f)
    for tap in range(11):
        w1w[tap % 5, tap // 5, :] = wc1[:, tap]
    w1w = b(w1w)
    w2w = b(np.asarray(inputs["wc2"], f).reshape(128, 64, 5)
            .transpose(1, 2, 0))
    w3w = b(np.asarray(inputs["wc3"], f).reshape(256, 128, 5)
            .transpose(1, 2, 0).reshape(128, 5, 2, 128))
    wb3w = np.ascontiguousarray(
        np.asarray(inputs["wb3"], f).reshape(2, 128).T)
    w4wT = b(np.asarray(inputs["wc4"], f).reshape(256, 256, 5)
             .transpose(1, 2, 0).reshape(2, 128, 5, 256)
             .transpose(1, 0, 2, 3))
    wb4row = b(np.asarray(inputs["wb4"], f).reshape(1, 256))
    one50 = b(np.ones((1, 50), f))

    def gru_prep(d):
        wih = np.asarray(inputs[f"wih_{d}"], f) / 512.0
        whh = np.asarray(inputs[f"whh_{d}"], f)
        bih = np.asarray(inputs[f"bih_{d}"], f)
        bhh = np.asarray(inputs[f"bhh_{d}"], f)
        wihT = np.ascontiguousarray(
            wih.reshape(3, 128, 4, 128).transpose(3, 0, 2, 1))
        whhT = np.ascontiguousarray(
            whh.reshape(3, 128, 128).transpose(2, 0, 1))
        brz = (bih + bhh)[:256].reshape(2, 128).T
        return wihT, whhT, brz, bih[256:], bhh[256:]

    wihT_f, whhT_f, brz_f, bn_f, bhn_f = gru_prep("f")
    wihT_b, whhT_b, brz_b, bn_b, bhn_b = gru_prep("b")
    wihT = b(np.stack([wihT_f, wihT_b], axis=1))
    whhT = b(np.stack([whhT_f, whhT_b], axis=1))
    brz = np.ascontiguousarray(np.stack([brz_f, brz_b], axis=1))
    brz[:, :, 1] *= -1.0
    bng = np.ascontiguousarray(np.stack([bn_f, bn_b], axis=1))
    bhhn = np.ascontiguousarray(np.stack([bhn_f, bhn_b], axis=1))
    clsW = np.asarray(inputs["clsW"], f) / 32.0
    clsT = b(clsW.reshape(5, 2, 128).transpose(2, 1, 0))
    clsb5 = np.asarray(inputs["clsb"], f).reshape(5, 1)

    shared = dict(
        wP=wP, w1m=w1m, bn1s=s1.reshape(64, 1), bn1b=b1.reshape(64, 1),
        w2m=b(w2m), bn2s=s2.reshape(128, 1), bn2b=b2.reshape(128, 1),
        w3m=w3m, b3=b3, fcm=fcm, fcwd=fcwd, nfb4=nfb4,
        w1w=w1w, wb1=np.asarray(inputs["wb1"], f).reshape(64, 1),
        w2w=w2w, wb2=np.asarray(inputs["wb2"], f).reshape(128, 1),
        w3w=w3w, wb3w=wb3w, w4wT=w4wT, wb4row=wb4row, one50=one50,
        wihT=wihT, brz=brz, bng=bng, bhhn=bhhn, whhT=whhT,
        clsT=clsT, clsb5=clsb5, zed=np.zeros((128, 2), f),
    )
    in_maps = []
    for i in range(NCORES):
        m = dict(shared)
        lms = lmp[:, 128 * i:128 * i + 136]
        lmI = np.empty((9, 8576), f)
        for dy in range(3):
            for dx in range(3):
                lmI[3 * dy + dx] = lms[dy:dy + 64, dx:dx + 134].reshape(-1)
        m["lmI"] = b(lmI)
        m["Ri"] = b(Rp[:, 64 * i:64 * i + 66])
        mk = np.ones((128, 2), f)
        if i == 0:
            mk[:, 0] = 0.0
        if i == NCORES - 1:
            mk[:, 1] = 0.0
        m["msk"] = mk
        in_maps.append(m)
    return in_maps


def kernel(**inputs):
    global LAST_RESULTS
    _ensure_concourse()
    from concourse import bass_utils

    if "nc" not in _CACHE:
        _CACHE["nc"] = _build()
    nc = _CACHE["nc"]
    in_maps = _prep_inputs(inputs)
    res = bass_utils.run_bass_kernel_spmd(
        nc, in_maps, core_ids=list(range(NCORES)))
    LAST_RESULTS = res
    return res.results[0]["out"]


if __name__ == "__main__":
    _ensure_concourse()
    _build()
    print("build + compile OK")
